# revision 43
# baseline (speedup 1.0000x reference)
"""Trainium2 Bass kernel for causal multi-head attention with roll-RoPE.

Problem: B=2, T=2048, C=1024, H=16 heads, HD=64.
  qkv = x @ W_attn; roll-RoPE on q,k; causal softmax attention; y @ W_proj.

Sharding: 8 cores; core c handles batch b=c//4 and head group g=c%4
(4 heads). Each core computes its partial output projection
y_heads @ W_proj[head_rows, :]; host sums the 4 partials per batch.

Device dataflow (per core):
  - x^T built by PE-transposing natural x tiles (two T-halves), f32r.
  - QKV^T in f32r: q/k produced transposed [hd, T] and stored bf16
    (pairs of heads stacked into 128 partitions), v natural [T, hd]
    f32r with an interleaved ones column (softmax denominators).
    1/sqrt(hd) folded into W_q host-side.
  - RoPE roll along hd (= partition shift by 1 within each 64-block) is
    a PE matmul with a block-diagonal bf16 permutation matrix; then
    q*cos + roll(q)*sin in bf16 (fast DVE modes) + one Pool PSUM evac.
  - Scores S^T = K^T.T @ Q^T (bf16 inputs, f32 PSUM) per (kb, qchunk),
    two heads row-tiled into one PE pass; exp on ACT; causal masking by
    zeroing the below-diag triangle of P with gpsimd affine_select.
  - Attention for pair 0, q-chunks 0/1 is interleaved into the second
    transpose half (it only needs half-0 roped q/k and v-tiles 0..7),
    which spreads the ACT exp work across most of the kernel.
  - O^T = (P @ [V|1])^T accumulated in PSUM over kb; partition 64 holds
    the softmax denominator. Normalize: DVE reciprocal in-place on the
    PSUM denominator row, gpsimd partition_broadcast, then multiply;
    the second head of each pair moves to partitions 64:128 via an
    SBUF->SBUF DMA; each pair's normalize is deferred into the next
    attention's instruction stream so its PE broadcast never stalls.
  - Projection: out_partial = y^T.T @ W_proj chunks, accumulated in
    PSUM over the two 128-row chunks; partials stored to HBM in bf16
    (host accumulates in f32).
  - Weight/cos/sin/perm loads are issued from the Pool engine (SWDGE)
    so the SP queue serves only x loads and out stores.
"""
import contextlib
import itertools
import sys

for _p in ("/opt/trn_rl_repo",):
    if _p not in sys.path:
        sys.path.insert(0, _p)

import numpy as np
import ml_dtypes

import concourse.bass as bass
import concourse.bacc as bacc
import concourse.tile as tile
import concourse.mybir as mybir
from concourse.bass_utils import run_bass_kernel_spmd
B, T, C, H, HD = 2, 2048, 1024, 16, 64
NCORES = 8
HPC = H // (NCORES // B)  # 4 heads per core
CC = C // 128             # 8 contraction chunks
TB = T // 128             # 16 t-blocks
QC = T // 512             # 4 q-chunks
F32 = mybir.dt.float32
F32R = mybir.dt.float32r
BF16 = mybir.dt.bfloat16


def r(ap):
    return ap.bitcast(F32R)


def build_program(bench_iters=None):
    nc = bacc.Bacc("TRN2", target_bir_lowering=False, debug=False)

    xb = nc.dram_tensor("xb", [T, C], F32R, kind="ExternalInput")
    wqk = nc.dram_tensor("wqk", [C, 4 * 128], F32R, kind="ExternalInput")
    wv = nc.dram_tensor("wv", [C, HPC * HD], F32R, kind="ExternalInput")
    wp = nc.dram_tensor("wp", [HPC * HD, C], F32R, kind="ExternalInput")
    cos2 = nc.dram_tensor("cos2", [128, T], BF16, kind="ExternalInput")
    sin2 = nc.dram_tensor("sin2", [128, T], BF16, kind="ExternalInput")
    perm = nc.dram_tensor("perm", [128, 128], BF16, kind="ExternalInput")
    identd = nc.dram_tensor("identd", [128, 128], F32R, kind="ExternalInput")
    ones65 = nc.dram_tensor("ones65", [65, 64], F32R, kind="ExternalInput")
    out = nc.dram_tensor("out", [T, C], BF16, kind="ExternalOutput")

    with tile.TileContext(nc) as tc:
        with tc.tile_pool(name="const", bufs=1) as constp, \
             tc.tile_pool(name="xnat", bufs=2) as xnatp, \
             tc.tile_pool(name="xT", bufs=1) as xTp, \
             tc.tile_pool(name="qk", bufs=1) as qkp, \
             tc.tile_pool(name="shift", bufs=2) as shiftp, \
             tc.tile_pool(name="v", bufs=1) as vp, \
             tc.tile_pool(name="p", bufs=5) as pp, \
             tc.tile_pool(name="yT", bufs=1) as yTp, \
             tc.tile_pool(name="small", bufs=2) as smallp, \
             tc.tile_pool(name="osb", bufs=2) as osbp, \
             tc.tile_pool(name="ps", bufs=3, space="PSUM") as psp, \
             tc.tile_pool(name="pso", bufs=2, space="PSUM") as psop:

            ident = constp.tile([128, 128], F32R, tag="ident")
            nc.gpsimd.dma_start(ident[:], identd.ap())
            ones65_sb = constp.tile([65, 64], F32R, tag="ones65")
            nc.gpsimd.dma_start(ones65_sb[:], ones65.ap())
            ones4 = constp.tile([128, 4], F32, tag="ones4")
            nc.vector.memset(ones4[:], 1.0)
            wqk_sb = constp.tile([128, CC, 4 * 128], F32R, tag="wqk")
            wv_sb = constp.tile([128, CC, HPC * HD], F32R, tag="wv")
            wp_sb = constp.tile([128, 2, C], F32R, tag="wp")
            perm_sb = constp.tile([128, 128], BF16, tag="perm")
            cos_sb = constp.tile([128, T], BF16, tag="cos")
            sin_sb = constp.tile([128, T], BF16, tag="sin")
            bench_ctx = (tc.For_i(0, bench_iters, 1)
                         if bench_iters else contextlib.nullcontext())
            bench_ctx.__enter__()
            # first x tile alone (PE transposes gate on it), then pairs,
            # alternating issue across SP/ACT/DVE queues (DMA cost is
            # issuing-engine occupancy; transfers overlap across queues).
            # All const loads go upfront on the Pool queue in first-use
            # order: they run concurrently with the x loads.
            xnat0 = xnatp.tile([128, 1, C], F32R, tag="xnat0", name="xnat0")
            nc.sync.dma_start(
                xnat0[:], xb.ap()[0:128, :].rearrange("(i p) c -> p i c", p=128))
            nc.gpsimd.dma_start(
                wv_sb[:], wv.ap().rearrange("(c p) m -> p c m", p=128))
            for j4 in range(4):
                nc.gpsimd.dma_start(
                    wqk_sb[:, :, j4 * 128:(j4 + 1) * 128],
                    wqk.ap()[:, j4 * 128:(j4 + 1) * 128].rearrange(
                        "(c p) m -> p c m", p=128))
            nc.gpsimd.dma_start(cos_sb[:], cos2.ap())
            nc.gpsimd.dma_start(sin_sb[:], sin2.ap())
            nc.gpsimd.dma_start(perm_sb[:], perm.ap())
            nc.gpsimd.dma_start(
                wp_sb[:], wp.ap().rearrange("(c p) m -> p c m", p=128))

            # qk[j] tiles: j=0 Q^T pair0, j=1 K^T pair0, j=2 Q^T pair1, j=3 K^T pair1
            # rows 0:64 = first head of pair, 64:128 = second head.
            qk = [qkp.tile([128, T], BF16, tag=f"qk{j}", name=f"qk{j}")
                  for j in range(4)]
            v_tiles = [vp.tile([128, HPC * 65], BF16, tag=f"v{tb}", name=f"v{tb}")
                       for tb in range(TB)]
            yT = [yTp.tile([128, T], F32R, tag=f"yT{p}", name=f"yT{p}")
                  for p in range(2)]

            # ---- phase A+B per T-half: x^T, then QKV^T and V ----
            # xT layout: [128, c(8) x t(1024)] -> col c*1024 + t
            # double-buffered per T-half so th1 transposes don't wait on
            # th0's consumers (QK matmuls)
            xTt = [xTp.tile([128, CC * 1024], F32R, tag=f"xT{th}",
                            name=f"xTt{th}") for th in range(2)]
            def emit_v(gtb):
                xT = xTt[gtb // 8]
                ps_v = psp.tile([128, HPC * HD], F32, tag="s",
                                name=f"psv{gtb}")
                for c in range(CC):
                    nc.tensor.matmul(ps_v[:],
                                     xT[:, c * 1024 + gtb % 8 * 128:c * 1024 + (gtb % 8 + 1) * 128],
                                     wv_sb[:, c],
                                     start=(c == 0), stop=(c == CC - 1))
                vt = v_tiles[gtb]
                v3 = vt[:].rearrange("p (h e) -> p h e", e=65)
                nc.vector.tensor_copy(
                    v3[:, :, 0:64],
                    ps_v[:].rearrange("p (h e) -> p h e", e=64))
                nc.gpsimd.tensor_copy(
                    v3[:, :, 64:65],
                    ones4[:].rearrange("p (h e) -> p h e", e=1))

            def emit_rope(j, hf):
                hs = slice(hf * 1024, (hf + 1) * 1024)
                ps_sh = psp.tile([128, 1024], F32, tag="s",
                                 name=f"psh{j}_{hf}")
                for n in range(2):
                    nc.tensor.matmul(
                        ps_sh[:, n * 512:(n + 1) * 512],
                        perm_sb[:],
                        qk[j][:, hf * 1024 + n * 512:hf * 1024 + (n + 1) * 512],
                        start=True, stop=True)
                sh = shiftp.tile([128, 1024], BF16, tag="shift",
                                 name=f"sh{j}_{hf}")
                nc.vector.scalar_tensor_tensor(
                    sh[:], ps_sh[:], 1.0, sin_sb[:, hs],
                    mybir.AluOpType.mult, mybir.AluOpType.mult)
                nc.gpsimd.tensor_mul(qk[j][:, hs], qk[j][:, hs],
                                     cos_sb[:, hs])
                nc.vector.tensor_add(qk[j][:, hs], qk[j][:, hs], sh[:])

            def emit_qk(th, j):
                ps_qk = psp.tile([128, 1024], F32, tag="s",
                                 name=f"psqk{th}_{j}")
                for c in range(CC):
                    for n in range(2):
                        nc.tensor.matmul(
                            ps_qk[:, n * 512:(n + 1) * 512],
                            wqk_sb[:, c, j * 128:(j + 1) * 128],
                            xTt[th][:, c * 1024 + n * 512:c * 1024 + (n + 1) * 512],
                            start=(c == 0), stop=(c == CC - 1))
                if j % 2 == 0:
                    nc.vector.tensor_copy(qk[j][:, th * 1024:(th + 1) * 1024],
                                          ps_qk[:])
                else:
                    nc.scalar.copy(qk[j][:, th * 1024:(th + 1) * 1024],
                                   ps_qk[:])

            def emit_proj(tb, single=False):
                ps_out = psp.tile([128, 1024], F32, tag="s", name=f"psout{tb}")
                for ch in range(2):
                    for nn in range(2):
                        nc.tensor.matmul(
                            ps_out[:, nn * 512:(nn + 1) * 512],
                            yT[ch][:, tb * 128:(tb + 1) * 128],
                            wp_sb[:, ch, nn * 512:(nn + 1) * 512],
                            start=(ch == 0), stop=(ch == 1))
                if single:
                    # tail: two parallel evac->store half-chains per block
                    o_sb = osbp.tile([128, 1, C], BF16, tag="osb")
                    orow = out.ap()[tb * 128:(tb + 1) * 128, :]
                    nc.vector.tensor_copy(o_sb[:, 0, 0:512], ps_out[:, 0:512])
                    nc.sync.dma_start(orow[:, 0:512], o_sb[:, 0, 0:512])
                    nc.scalar.copy(o_sb[:, 0, 512:1024],
                                   ps_out[:, 512:1024])
                    nc.scalar.dma_start(orow[:, 512:1024],
                                        o_sb[:, 0, 512:1024])
                    return
                if tb % 2 == 0:
                    emit_proj.cur_osb = osbp.tile([128, 2, C], BF16, tag="osb")
                o_sb = emit_proj.cur_osb
                half = tb % 2
                if tb % 4 == 3:
                    nc.scalar.copy(o_sb[:, half, :], ps_out[:])
                else:
                    nc.vector.tensor_copy(o_sb[:, half, :], ps_out[:])
                if half == 1:
                    nc.sync.dma_start(
                        out.ap()[(tb - 1) * 128:(tb + 1) * 128, :].rearrange(
                            "(i p) c -> p i c", p=128),
                        o_sb[:])

            norm_pend = [None]

            def emit_norm(p, qc, ps_o, pe_move):
                # normalize: y^T = O^T * (1/denom), denom at partition 64.
                qcs = slice(qc * 512, (qc + 1) * 512)
                for h01 in (1, 0):
                    rcp_t = smallp.tile([65, 512], F32R, tag="rcp")
                    with nc.allow_low_precision(reason="f32 recip, 1e-4 ok"):
                        nc.vector.reciprocal(rcp_t[64:65, :],
                                             ps_o[h01][64:65, :])
                    # broadcast 1/denom across partitions via K=1 matmul
                    ps_rep = psp.tile([64, 512], F32, tag="s",
                                      name=f"psrep{p}_{qc}_{h01}")
                    nc.tensor.matmul(ps_rep[:], ones65_sb[64:65, :],
                                     rcp_t[64:65, :], start=True, stop=True,
                                     tile_position=(64, 0))
                    den = smallp.tile([64, 512], F32, tag="den")
                    nc.scalar.copy(den[:], ps_rep[:])
                    if h01 == 0:
                        nc.vector.tensor_mul(yT[p][0:64, qcs],
                                             ps_o[0][0:64, :], den[0:64, :])
                    else:
                        tmp = smallp.tile([64, 512], F32R, tag="tmp")
                        nc.vector.tensor_mul(tmp[:], ps_o[1][0:64, :],
                                             den[0:64, :])
                        nc.sync.dma_start(yT[p][64:128, qcs], tmp[:])

            def att_gen(p, qc, pe_move=False, final=False):
                """Generator: one yield per kb step so attention can be
                interleaved with other PE work at emission granularity."""
                q_t, k_t = qk[2 * p], qk[2 * p + 1]
                ps_o = [psop.tile([65, 512], F32, tag="o",
                                  name=f"pso{p}_{qc}_{i}")
                        for i in range(2)]
                nkb = 4 * qc + 4
                def emit_av(kb, p_t, col0):
                    for h01 in range(2):
                        h = 2 * p + h01
                        nc.tensor.matmul(
                            ps_o[h01][:, col0:512],
                            v_tiles[kb][:, h * 65:(h + 1) * 65],
                            p_t[:, h01 * 512 + col0:h01 * 512 + 512],
                            start=(kb == 0), stop=(kb == nkb - 1),
                            skip_group_check=True)

                pend_av = []  # software pipeline: AV lags S/exp by two kb
                for kb in range(nkb):
                    rr = kb - 4 * qc  # >=0 on diagonal blocks
                    diag = rr >= 0
                    col0_mm = 0 if rr < 0 else rr * 128
                    col0_ex = col0_mm
                    ncols = 512 - col0_mm
                    qs = slice(qc * 512 + col0_mm, (qc + 1) * 512)
                    ks = slice(kb * 128, (kb + 1) * 128)
                    ps_s = psp.tile([128, 1024], F32, tag="s",
                                    name=f"pss{p}_{qc}_{kb}")
                    nc.tensor.matmul(ps_s[:, 0:ncols],
                                     k_t[0:64, ks], q_t[0:64, qs],
                                     start=True, stop=True,
                                     skip_group_check=True,
                                     tile_position=(0, 0))
                    nc.tensor.matmul(ps_s[:, 512:512 + ncols],
                                     k_t[64:128, ks], q_t[64:128, qs],
                                     start=True, stop=True,
                                     skip_group_check=True,
                                     tile_position=(64, 0))
                    p_t = pp.tile([128, 1024], BF16, tag="p")
                    p3 = p_t[:].rearrange("p (h e) -> p h e", e=512)
                    nc.scalar.activation(
                        p3[:, :, col0_ex:512],
                        ps_s[:].rearrange("p (h e) -> p h e", e=512)[
                            :, :, col0_ex - col0_mm:ncols],
                        mybir.ActivationFunctionType.Exp)
                    if diag:
                        # zero below-diagonal triangle: keep where j>=p
                        nc.gpsimd.affine_select(
                            p3[:, :, col0_ex:col0_ex + 128],
                            p3[:, :, col0_ex:col0_ex + 128],
                            pattern=[[0, 2], [1, 128]],
                            compare_op=mybir.AluOpType.is_ge,
                            fill=0.0, base=0, channel_multiplier=-1)
                    pend_av.append((kb, p_t, col0_mm))
                    if len(pend_av) > 3:
                        emit_av(*pend_av.pop(0))
                    yield
                    if kb == 1 and norm_pend[0] is not None:
                        emit_norm(*norm_pend[0])
                        norm_pend[0] = None
                while pend_av:
                    emit_av(*pend_av.pop(0))
                # defer this att's normalize into the next att's stream
                # (its PE broadcast then sits behind independent S work)
                norm_pend[0] = (p, qc, ps_o, pe_move)
                if final:
                    emit_norm(*norm_pend[0])
                    norm_pend[0] = None
                yield

            def drive(gen, n):
                """Advance gen by n steps (or to exhaustion)."""
                for _ in itertools.islice(gen, n):
                    pass

            pend_v = []
            att01 = None
            # x load schedule: first tile alone, then pairs, pipelined
            # ahead of the transposes that consume them; issue engine
            # rotates so no single queue serializes the loads
            loads = {(0, 0): ([1], [2, 3]), (0, 1): ([4, 5],),
                     (0, 2): ([6, 7],), (0, 4): ([8, 9],),
                     (1, 0): ([10, 11],), (1, 2): ([12, 13],),
                     (1, 4): ([14, 15],)}
            load_eng = {(0, 0): (nc.scalar, nc.sync), (0, 1): (nc.scalar,),
                        (0, 2): (nc.sync,), (0, 4): (nc.gpsimd,),
                        (1, 0): (nc.sync,), (1, 2): (nc.scalar,),
                        (1, 4): (nc.sync,)}
            xmap = {0: (xnat0, 0)}
            for th in range(2):
                for tb in range(8):
                    gtb = th * 8 + tb
                    for grp, eng in zip(loads.get((th, tb), ()),
                                        load_eng.get((th, tb), ())):
                        xn = xnatp.tile([128, len(grp), C], F32R, tag="xnat")
                        eng.dma_start(
                            xn[:],
                            xb.ap()[grp[0] * 128:(grp[-1] + 1) * 128, :].rearrange(
                                "(i p) c -> p i c", p=128))
                        for i, g2 in enumerate(grp):
                            xmap[g2] = (xn, i)
                    xnat, xi = xmap[gtb]
                    pst = psp.tile([128, 1024], F32, tag="s")
                    for c in range(CC):
                        nc.tensor.transpose(
                            pst[:, c * 128:(c + 1) * 128].bitcast(F32R),
                            xnat[:, xi, c * 128:(c + 1) * 128],
                            ident[:])
                    xt_dst = xTt[th][:].rearrange("p (c t) -> p c t", t=1024)[:, :, tb * 128:(tb + 1) * 128]
                    xt_src = pst[:].rearrange("p (c t) -> p c t", t=128)
                    if th == 1 and tb % 2 == 0:
                        nc.scalar.copy(xt_dst, xt_src)
                    else:
                        nc.vector.tensor_copy(xt_dst, xt_src)
                    # V fills PE while the next transposes wait on evac;
                    # lagged one tb (avoids head-of-line blocking the
                    # in-order PE queue on the evac)
                    while pend_v:
                        emit_v(pend_v.pop(0))
                    pend_v.append(gtb)
                    if th == 1 and tb >= 1:
                        # attention pair0 qc0/qc1 interleaves with the
                        # second transpose half: 2 kb-steps per tb
                        drive(att01, 2)
                if th == 0:
                    while pend_v:
                        emit_v(pend_v.pop(0))
                    emit_qk(0, 0)
                    emit_qk(0, 1)
                    emit_rope(0, 0)
                    emit_rope(1, 0)
                    emit_qk(0, 2)
                    emit_qk(0, 3)
                    att01 = itertools.chain(att_gen(0, 0), att_gen(0, 1))
            while pend_v:
                emit_v(pend_v.pop(0))
            drive(att01, 99)  # finish any remainder
            emit_qk(1, 0)
            emit_qk(1, 1)
            emit_rope(0, 1)
            emit_rope(1, 1)
            # att(0,2) hides the DVE/Pool rope work of pair 1
            g = att_gen(0, 2)
            drive(g, 3); emit_qk(1, 2)
            drive(g, 3); emit_qk(1, 3)
            drive(g, 2); emit_rope(2, 0)
            drive(g, 2); emit_rope(3, 0)
            drive(g, 2); emit_rope(2, 1)
            drive(g, 1); emit_rope(3, 1)
            drive(g, 99)
            nc.gpsimd.dma_start(
                wp_sb[:], wp.ap().rearrange("(c p) m -> p c m", p=128))
            drive(att_gen(1, 0), 99)
            g = att_gen(0, 3)
            drive(g, 9)
            for tb in range(0, 4):
                emit_proj(tb)
                drive(g, 2)
            drive(g, 99)
            drive(att_gen(1, 1), 99)
            g = att_gen(1, 2)
            drive(g, 5)
            for tb in range(4, 8):
                emit_proj(tb)
                drive(g, 2)
            drive(g, 99)
            g = att_gen(1, 3, final=True)
            drive(g, 9)
            for tb in range(8, 12):
                emit_proj(tb)
                drive(g, 2)
            drive(g, 99)
            for tb in range(12, 16):
                emit_proj(tb, single=True)
            bench_ctx.__exit__(None, None, None)

    nc.compile()
    return nc


def host_inputs(x, W_attn, W_proj):
    """Per-core input dicts. Folds 1/sqrt(HD) into W_q; builds cos/sin
    tables (bf16) and the roll permutation matrix."""
    x = np.asarray(x, dtype=np.float32)
    W_attn = np.asarray(W_attn, dtype=np.float32)
    W_proj = np.asarray(W_proj, dtype=np.float32)

    inv_freq = (1.0 / (10000.0 ** (np.arange(0, HD, 2, dtype=np.float32) / HD))
                ).astype(np.float32)
    freqs = (np.arange(T, dtype=np.float32)[:, None] * inv_freq[None, :]
             ).astype(np.float32)
    emb = np.concatenate([freqs, freqs], axis=-1)  # [T, HD]
    cosT = np.cos(emb).T.astype(np.float32)        # [HD, T]
    sinT = np.sin(emb).T.astype(np.float32)
    cos2 = np.concatenate([cosT, cosT], axis=0).astype(ml_dtypes.bfloat16)
    sin2 = np.concatenate([sinT, sinT], axis=0).astype(ml_dtypes.bfloat16)
    # roll-by-1 within each 64-partition block: out[p] = in[(p-1) % 64]
    src = np.concatenate([(np.arange(64) - 1) % 64,
                          (np.arange(64) - 1) % 64 + 64])
    perm = np.zeros((128, 128), dtype=np.float32)
    perm[src, np.arange(128)] = 1.0
    perm = perm.astype(ml_dtypes.bfloat16)
    identity = np.eye(128, dtype=np.float32)
    scale = np.float32(1.0 / np.sqrt(HD))

    in_maps = []
    for core in range(NCORES):
        b = core // (NCORES // B)
        g = core % (NCORES // B)
        h0 = g * HPC  # first global head of this core
        wqk = np.empty((C, 4 * 128), dtype=np.float32)
        for pair in range(2):
            ha = h0 + 2 * pair
            wqk[:, (2 * pair) * 128:(2 * pair) * 128 + 64] = \
                W_attn[:, ha * HD:(ha + 1) * HD] * scale
            wqk[:, (2 * pair) * 128 + 64:(2 * pair + 1) * 128] = \
                W_attn[:, (ha + 1) * HD:(ha + 2) * HD] * scale
            wqk[:, (2 * pair + 1) * 128:(2 * pair + 1) * 128 + 64] = \
                W_attn[:, C + ha * HD:C + (ha + 1) * HD]
            wqk[:, (2 * pair + 1) * 128 + 64:(2 * pair + 2) * 128] = \
                W_attn[:, C + (ha + 1) * HD:C + (ha + 2) * HD]
        wv = W_attn[:, 2 * C + h0 * HD:2 * C + (h0 + HPC) * HD].copy()
        wp = W_proj[h0 * HD:(h0 + HPC) * HD, :].copy()
        in_maps.append({
            "xb": np.ascontiguousarray(x[b]),
            "wqk": wqk,
            "wv": np.ascontiguousarray(wv),
            "wp": np.ascontiguousarray(wp),
            "cos2": cos2,
            "sin2": sin2,
            "perm": perm,
            "identd": identity,
            "ones65": np.ones((65, 64), dtype=np.float32),
        })
    return in_maps


_NC_CACHE = [None]


def get_nc():
    if _NC_CACHE[0] is None:
        _NC_CACHE[0] = build_program()
    return _NC_CACHE[0]


def kernel(x, W_attn, W_proj, mask=None):
    nc = get_nc()
    in_maps = host_inputs(x, W_attn, W_proj)
    res = run_bass_kernel_spmd(nc, in_maps, core_ids=list(range(NCORES)))
    out = np.zeros((B, T, C), dtype=np.float32)
    for core in range(NCORES):
        b = core // (NCORES // B)
        out[b] += np.asarray(res.results[core]["out"], dtype=np.float32)
    return out
